# revision 15
# baseline (speedup 1.0000x reference)
"""RGCNConv (mean aggr) + ReLU on 8 Trainium2 NeuronCores.

Strategy (dst-sharded SPMD, one NEFF for all 8 cores):
  - Nodes padded to 102400 = 8 cores x 12800; core k owns dst range
    [k*12800, (k+1)*12800) = 25 superwindows (sw) of 512 nodes.
  - Edges are routed to the core owning their dst. Mean divisor 1/c is
    host-folded into per-edge weights (index-derived preprocessing).
  - Gather of x[src] rows happens on-device via gpsimd dma_gather
    (int16 indices => 4 source buckets of 32768 rows; edge stream is
    ordered (sw, bucket, relation) and padded to 128-row tiles per
    (sw, bucket, r) cell so each tile maps to one (sw, r) group).
  - Segment-sum via selector matmuls: for each 128-edge tile,
    S[p, slot] = w_p * (slot == dst_p mod 512) built on DVE from an
    iota constant; PE computes psum_A[fin, 512] += G_tile^T-style
    (lhsT=G[edges,fin], rhs=S[edges,512]) accumulating all tiles of a
    (sw, r) group.
  - Phase 2: psum_msg[fout, 512] += W_r^T @ A_r for r=0..7 plus the
    root term W_root^T @ x_own^T (x^T supplied by host), then
    relu(psum_msg + bias) on the scalar engine, stored transposed.
  - Host reassembles: out = concat(per-core outT).T[:100000].
"""

import numpy as np
import ml_dtypes

N_NODES = 100000
N_EDGES = 600000
D = 128
R = 8
N_CORES = 8
PER_CORE = 12800          # 25 * 512
N_PAD = PER_CORE * N_CORES
SW = 512                  # superwindow (PSUM slot width)
NSW = PER_CORE // SW      # 25
BUCKET = 25000            # balanced bucket size (< 32768 so int16 indices fit)
NBUCKET = 4               # ceil(100000 / 32768)
TILE = 128
MAX_IDX_PER_CALL = 1024   # dma_gather descriptor-ring safe size

_compiled = None          # (nc, plan_key, plan) cache


def _build_plan(src, dst, et, invc):
    """Partition/sort/pad edges. Returns per-core streams + shared tile map.

    Stream order: for sw in 0..24: for bucket in 0..3: for r in 0..7:
    cell edges (padded to a shared multiple of 128 rows per cell).
    """
    core = dst // PER_CORE
    local = dst - core * PER_CORE
    sw = local // SW
    slot = local % SW
    bucket = np.minimum(src // BUCKET, NBUCKET - 1).astype(np.int64)
    w = invc[dst * R + et]

    # per-core sort by (sw, bucket, r, dst)
    per_core = []
    cell_counts = np.zeros((N_CORES, NSW, NBUCKET, R), np.int64)
    for k in range(N_CORES):
        m = core == k
        order = np.lexsort((local[m], et[m], bucket[m], sw[m]))
        sk = src[m][order]
        slk = slot[m][order]
        swk = sw[m][order]
        bk = bucket[m][order]
        rk = et[m][order]
        wk = w[m][order]
        per_core.append((sk, slk, swk, bk, rk, wk))
        np.add.at(cell_counts[k], (swk, bk, rk), 1)

    # shared tile budget per cell
    tiles_cell = np.ceil(cell_counts.max(axis=0) / TILE).astype(np.int64)  # [NSW, NB, R]

    # shared layout: tile index of each cell, call spans per (sw, bucket)
    cell_tile_start = np.zeros((NSW, NBUCKET, R), np.int64)
    t = 0
    section_spans = []  # (sw, bucket, tile_start, n_tiles)
    for s in range(NSW):
        for b in range(NBUCKET):
            sec_start = t
            for r in range(R):
                cell_tile_start[s, b, r] = t
                t += tiles_cell[s, b, r]
            section_spans.append((s, b, sec_start, t - sec_start))
    total_tiles = t

    # per-core padded streams
    streams = []
    for k in range(N_CORES):
        sk, slk, swk, bk, rk, wk = per_core[k]
        idx16 = np.zeros(total_tiles * TILE, np.int16)
        slot_f = np.zeros(total_tiles * TILE, np.float32)
        w_f = np.zeros(total_tiles * TILE, np.float32)
        # position of each real edge: cell-major offset
        # edges are already sorted by (sw, bucket, r); compute within-cell rank
        cellid = (swk * NBUCKET + bk) * R + rk
        # rank within cell
        change = np.empty(len(cellid), bool)
        change[0] = True
        if len(cellid) > 1:
            change[1:] = cellid[1:] != cellid[:-1]
        grp_start = np.flatnonzero(change)
        rank = np.arange(len(cellid)) - np.repeat(grp_start, np.diff(np.append(grp_start, len(cellid))))
        pos = cell_tile_start[swk, bk, rk] * TILE + rank
        idx16[pos] = (sk - bk * BUCKET).astype(np.int16)
        slot_f[pos] = slk.astype(np.float32)
        w_f[pos] = wk.astype(np.float32)
        # pads (w==0): duplicate the previous real edge's source row so the
        # padded gather hits a hot HBM row instead of hammering row 0.
        real = w_f != 0.0
        ffill = np.maximum.accumulate(np.where(real, np.arange(len(idx16)), -1))
        has_prev = ffill >= 0
        idx16[~real & has_prev] = idx16[ffill[~real & has_prev]]
        streams.append((idx16, slot_f, w_f))

    return tiles_cell, cell_tile_start, section_spans, total_tiles, streams


def _build_bass(tiles_cell, cell_tile_start, section_spans, total_tiles, dt_str):
    import concourse.bass as bass
    import concourse.bacc as bacc
    import concourse.mybir as mybir
    import concourse.tile as tile
    from concourse import library_config

    DT = {"bf16": mybir.dt.bfloat16, "fp16": mybir.dt.float16,
          "fp32": mybir.dt.float32}[dt_str]
    # slot/w scalar streams must be fp32 (is_equal requires fp32 scalar);
    # iota is the wide in0 operand - fp16 keeps the DVE 2x/4x perf modes.
    MDT = mybir.dt.float32
    IDT = mybir.dt.float16 if dt_str == "fp16" else mybir.dt.float32
    TOT = total_tiles * TILE

    nc = bacc.Bacc("TRN2", target_bir_lowering=False, debug=False, num_devices=1,
                   num_swdge_queues=4)
    xb_d = nc.dram_tensor("xb", [N_NODES, D], DT, kind="ExternalInput").ap()
    xt_d = nc.dram_tensor("xt", [D, PER_CORE], DT, kind="ExternalInput").ap()
    idx_d = nc.dram_tensor("idx", [128, TOT // 16], mybir.dt.int16, kind="ExternalInput").ap()
    slot_d = nc.dram_tensor("slot", [128, total_tiles], MDT, kind="ExternalInput").ap()
    w_d = nc.dram_tensor("w", [128, total_tiles], MDT, kind="ExternalInput").ap()
    ws_d = nc.dram_tensor("ws", [D, (R + 1) * D], DT, kind="ExternalInput").ap()
    iota_d = nc.dram_tensor("iota", [128, SW], IDT, kind="ExternalInput").ap()
    bias_d = nc.dram_tensor("bias", [128, 1], mybir.dt.float32, kind="ExternalInput").ap()
    out_d = nc.dram_tensor("outT", [D, PER_CORE], DT, kind="ExternalOutput").ap()

    nc.gpsimd.load_library(library_config.mlp)

    # tiles of each (sw, r) group in stream order
    group_tiles = {}
    for s in range(NSW):
        for r in range(R):
            lst = []
            for b in range(NBUCKET):
                t0 = cell_tile_start[s, b, r]
                lst.extend(range(t0, t0 + tiles_cell[s, b, r]))
            group_tiles[(s, r)] = lst

    # max rows of any one-pass staging (pass = one sw)
    sw_rows = [int(tiles_cell[s].sum()) * TILE for s in range(NSW)]
    max_sw_rows = max(sw_rows)

    with tile.TileContext(nc) as tc:
        with (
            tc.tile_pool(name="meta", bufs=1) as meta,
            tc.tile_pool(name="stage", bufs=3) as stage_pool,
            tc.tile_pool(name="sbuf_s", bufs=6) as s_pool,
            tc.tile_pool(name="sbuf_a", bufs=4) as a_pool,
            tc.tile_pool(name="sbuf_xr", bufs=3) as xr_pool,
            tc.tile_pool(name="sbuf_o", bufs=3) as o_pool,
            tc.tile_pool(name="psum_a", bufs=2, space="PSUM") as pa_pool,
            tc.tile_pool(name="psum_m", bufs=2, space="PSUM") as pm_pool,
        ):
            idx_sb = meta.tile([128, TOT // 16], mybir.dt.int16)
            nc.sync.dma_start(idx_sb[:], idx_d[:])
            slot_sb = meta.tile([128, total_tiles], MDT)
            nc.sync.dma_start(slot_sb[:], slot_d[:])
            w_sb = meta.tile([128, total_tiles], MDT)
            nc.sync.dma_start(w_sb[:], w_d[:])
            ws_sb = meta.tile([D, (R + 1) * D], DT)
            nc.sync.dma_start(ws_sb[:], ws_d[:])
            iota_sb = meta.tile([128, SW], IDT)
            nc.sync.dma_start(iota_sb[:], iota_d[:])
            bias_sb = meta.tile([128, 1], mybir.dt.float32)
            nc.sync.dma_start(bias_sb[:], bias_d[:])

            for s in range(NSW):
                rows_s = sw_rows[s]
                if rows_s == 0:
                    continue
                sw_tile0 = int(cell_tile_start[s, 0, 0])
                stg = stage_pool.tile([128, max_sw_rows], DT, tag="stage")
                # gather this sw's sections (one or more calls per bucket)
                for (s2, b, sec_t0, sec_nt) in section_spans:
                    if s2 != s or sec_nt == 0:
                        continue
                    n_rows = sec_nt * TILE
                    done = 0
                    while done < n_rows:
                        n = min(MAX_IDX_PER_CALL, n_rows - done)
                        row0 = (sec_t0 - sw_tile0) * TILE + done
                        glob0 = sec_t0 * TILE + done
                        nc.gpsimd.dma_gather(
                            out_ap=stg[:, row0:row0 + n].rearrange(
                                "p (t f) -> p t f", f=TILE),
                            in_ap=xb_d[b * BUCKET:(b + 1) * BUCKET, :],
                            idxs_ap=idx_sb[:, glob0 // 16:(glob0 + n) // 16],
                            num_idxs=n,
                            num_idxs_reg=n,
                            elem_size=D,
                            queue_num=b,
                        )
                        done += n

                psum_m = pm_pool.tile([128, SW], mybir.dt.float32, space="PSUM")
                first_mm = True
                for r in range(R):
                    tlist = group_tiles[(s, r)]
                    if not tlist:
                        continue
                    psum_a = pa_pool.tile([128, SW], mybir.dt.float32, space="PSUM")
                    for i, t in enumerate(tlist):
                        s_sb = s_pool.tile([128, SW], DT, tag="sel")
                        nc.vector.tensor_scalar(
                            out=s_sb[:], in0=iota_sb[:],
                            scalar1=slot_sb[:, t:t + 1],
                            op0=mybir.AluOpType.is_equal,
                            scalar2=None,
                        )
                        lrow = (t - sw_tile0) * TILE
                        # fold per-edge weight into the gathered rows
                        # (keeps the 512-wide selector build single-pass)
                        nc.vector.tensor_scalar(
                            out=stg[:, lrow:lrow + TILE],
                            in0=stg[:, lrow:lrow + TILE],
                            scalar1=w_sb[:, t:t + 1],
                            op0=mybir.AluOpType.mult,
                            scalar2=None,
                        )
                        nc.tensor.matmul(
                            out=psum_a[:],
                            lhsT=stg[:, lrow:lrow + TILE],
                            rhs=s_sb[:],
                            start=(i == 0), stop=(i == len(tlist) - 1),
                        )
                    a_sb = a_pool.tile([128, SW], DT, tag="acc")
                    nc.scalar.copy(a_sb[:], psum_a[:])
                    nc.tensor.matmul(
                        out=psum_m[:],
                        lhsT=ws_sb[:, r * D:(r + 1) * D],
                        rhs=a_sb[:],
                        start=first_mm, stop=False,
                    )
                    first_mm = False
                # root term
                xr_sb = xr_pool.tile([128, SW], DT, tag="xr")
                nc.sync.dma_start(xr_sb[:], xt_d[:, s * SW:(s + 1) * SW])
                nc.tensor.matmul(
                    out=psum_m[:],
                    lhsT=ws_sb[:, R * D:(R + 1) * D],
                    rhs=xr_sb[:],
                    start=first_mm, stop=True,
                )
                o_sb = o_pool.tile([128, SW], DT, tag="out")
                nc.scalar.activation(
                    o_sb[:], psum_m[:],
                    mybir.ActivationFunctionType.Relu,
                    bias=bias_sb[:, 0:1],
                )
                nc.sync.dma_start(out_d[:, s * SW:(s + 1) * SW], o_sb[:])

    nc.compile()
    return nc


def kernel(x, edge_index, edge_type, W, root, bias, dt_str="fp16"):
    from concourse.bass_utils import run_bass_kernel_spmd

    global _compiled
    x = np.asarray(x, np.float32)
    src = np.asarray(edge_index[0], np.int64)
    dst = np.asarray(edge_index[1], np.int64)
    et = np.asarray(edge_type, np.int64)
    W = np.asarray(W, np.float32)
    root = np.asarray(root, np.float32)
    bias = np.asarray(bias, np.float32)

    counts = np.bincount(dst * R + et, minlength=N_NODES * R).astype(np.float32)
    invc = 1.0 / np.maximum(counts, 1.0)

    tiles_cell, cell_tile_start, section_spans, total_tiles, streams = _build_plan(
        src, dst, et, invc)

    plan_key = (dt_str, total_tiles, tiles_cell.tobytes())
    if _compiled is None or _compiled[0] != plan_key:
        nc = _build_bass(tiles_cell, cell_tile_start, section_spans,
                         total_tiles, dt_str)
        _compiled = (plan_key, nc)
    nc = _compiled[1]

    np_dt = {"bf16": ml_dtypes.bfloat16, "fp16": np.float16,
             "fp32": np.float32}[dt_str]
    np_mdt = np.float32
    np_idt = np.float16 if dt_str == "fp16" else np.float32
    xb = x.astype(np_dt)
    xt_pad = np.zeros((D, N_PAD), np.float32)
    xt_pad[:, :N_NODES] = x.T
    xt_pad = xt_pad.astype(np_dt)
    # layout [D fin, (R+1)*D] with W[r] at cols r*D:(r+1)*D, root last
    ws_flat = np.concatenate([W[r] for r in range(R)] + [root], axis=1).astype(np_dt)
    iota = np.tile(np.arange(SW, dtype=np_idt)[None, :], (128, 1))
    bias_in = bias[:, None].astype(np.float32)

    in_maps = []
    for k in range(N_CORES):
        idx16, slot_f, w_f = streams[k]
        TOT = total_tiles * TILE
        idx_wrapped = np.zeros((16, TOT // 16), np.int16)
        j = np.arange(TOT)
        idx_wrapped[j % 16, j // 16] = idx16
        idx_rep = np.tile(idx_wrapped, (8, 1))
        slot_arr = slot_f.reshape(total_tiles, TILE).T.astype(np_mdt)  # [128, T]
        w_arr = w_f.reshape(total_tiles, TILE).T.astype(np_mdt)
        in_maps.append({
            "xb": xb,
            "xt": np.ascontiguousarray(xt_pad[:, k * PER_CORE:(k + 1) * PER_CORE]),
            "idx": idx_rep,
            "slot": slot_arr,
            "w": w_arr,
            "ws": ws_flat,
            "iota": iota,
            "bias": bias_in,
        })

    res = run_bass_kernel_spmd(nc, in_maps, core_ids=list(range(N_CORES)))
    out = np.empty((N_PAD, D), np.float32)
    for k in range(N_CORES):
        out[k * PER_CORE:(k + 1) * PER_CORE] = res.results[k]["outT"].T.astype(np.float32)
    return out[:N_NODES]

